# revision 10
# baseline (speedup 1.0000x reference)
"""Causal multi-head attention (B=2, H=16, S=2048, D=64, fp32 I/O) on 8 TRN2
NeuronCores.

Sharding: batch*heads (32 units) split 4-per-core — embarrassingly parallel,
no collectives.

Per-core kernel v2 (bf16 compute, fp32 PSUM accumulation):
  - scores computed TRANSPOSED: scoresT[k, q] = K_blk @ Q^T; softmax
    numerators P^T feed P@V directly with V (+ones column for the softmax
    denominator) as the stationary operand, PADDED to 128 columns so the
    compiler's fast-weight-load (FWL) path triggers.
  - PSUM->SBUF score eviction is split across BOTH ScalarE and VectorE:
    ScalarE chunks use the exact Exp activation; VectorE chunks use a
    1-instruction Schraudolph exp (int16(x*A+B) bits reinterpreted as bf16,
    ~1.8% avg rel err on a ~25-40%% fraction of elements — global output
    rel err stays ~1e-2 < 2e-2). A static greedy balancer assigns chunks.
  - causal diag-block masking: VectorE chunks fuse the mask into the
    Schraudolph affine via a per-chunk bias tensor (-30k => ~-1e-32 weights);
    ScalarE chunks get a single merged GpSimd 0/1-mask multiply.
  - PSUM: 3 double-bank score slots + 1 double-bank out^T accumulator
    (deeper score pipeline hides PE latency; out^T is evicted per-bank
    early enough that single-buffering doesn't stall the next half).
  - K^T/Q^T built via ONE blocked DMA-transpose per q-half from a combined
    [K|Q-duplicated] bf16 buffer; next head's casts+transposes are emitted
    during the current head's second half so ScalarE never idles at head
    boundaries. A dozen dummy matmuls at startup pre-warm the PE HAM clock.
"""

import numpy as np

import concourse.bass as bass
import concourse.mybir as mybir
import concourse.tile as tile
from concourse import bacc
from concourse.bass_utils import run_bass_kernel_spmd
from concourse.masks import make_upper_triangular
from concourse.alu_op_type import AluOpType

B, H, S, D = 2, 16, 2048, 64
N_CORES = 8
HPC = (B * H) // N_CORES  # heads per core
NT = S // 128  # 16 k/q blocks of 128
FP32 = mybir.dt.float32
BF16 = mybir.dt.bfloat16
I16 = mybir.dt.int16

LOG2E = 1.4426950408889634
EXP_A = 128.0 * LOG2E / 8.0  # folds softmax scale 1/sqrt(64) into the affine
EXP_B = 127.0 * 128.0 - 5.5
MASK_NEG = -30720.0

SC_BUFS = 3
OPS_BUFS = 1
WARM_MMS = 12

# engine cost models (ns) for the static eviction balancer
def _act_cost(el):
    return (el + 230) * 0.8333


def _dve_cost(el):
    return (el + 140) * 1.0417


def build_attention():
    nc = bacc.Bacc("TRN2", target_bir_lowering=False)
    q_d = nc.dram_tensor("query", [HPC, S, D], FP32, kind="ExternalInput")
    k_d = nc.dram_tensor("key", [HPC, S, D], FP32, kind="ExternalInput")
    v_d = nc.dram_tensor("value", [HPC, S, D], FP32, kind="ExternalInput")
    o_d = nc.dram_tensor("out", [HPC, S, D], FP32, kind="ExternalOutput")

    T = {"act": 0.0, "dve": 0.0, "gps": 0.0}

    with tile.TileContext(nc) as tc:
        with (
            tc.tile_pool(name="singles", bufs=1) as singles,
            tc.tile_pool(name="nat", bufs=3) as nat_pool,
            tc.tile_pool(name="bf", bufs=2) as bf_pool,
            tc.tile_pool(name="slab", bufs=2) as slab_pool,
            tc.tile_pool(name="vp", bufs=2) as v_pool,
            tc.tile_pool(name="pt", bufs=9) as pt_pool,
            tc.tile_pool(name="ep", bufs=4) as ep_pool,
            tc.tile_pool(name="sc", bufs=SC_BUFS, space="PSUM") as sc_pool,
            tc.tile_pool(name="ops", bufs=OPS_BUFS, space="PSUM") as ops_pool,
        ):
            # ---- one-time masks ----
            # 0/1 keep-mask for ScalarE diag chunks (partition = k, free = q)
            tri01 = singles.tile([128, 128], BF16, tag="tri01")
            make_upper_triangular(nc, tri01, val=1.0, diag=True)
            # merged 2-lane mask: lane0 diag at cols 0:128, lane1 at 128:256
            m2 = singles.tile([128, 2, 256], BF16, tag="m2")
            nc.gpsimd.memset(m2, 1.0)
            nc.gpsimd.tensor_copy(m2[:, 0, 0:128], tri01)
            nc.gpsimd.tensor_copy(m2[:, 1, 128:256], tri01)
            # VectorE fast-exp bias-with-mask: B where keep, -30k where masked
            mbt = singles.tile([128, 128], FP32, tag="mbt")
            nc.gpsimd.memset(mbt, MASK_NEG)
            nc.gpsimd.affine_select(
                out=mbt,
                in_=mbt,
                compare_op=mybir.AluOpType.is_gt,
                fill=EXP_B,
                base=0,
                pattern=[[-1, 128]],
                channel_multiplier=1,
            )
            mb = singles.tile([128, 2, 512], FP32, tag="mb")
            nc.vector.memset(mb, EXP_B)
            nc.vector.tensor_copy(mb[:, 0, 0:128], mbt)
            nc.vector.tensor_copy(mb[:, 1, 128:256], mbt)

            # PE HAM warm-up: dummy matmuls so the clock gate opens before
            # the first real QK^T (they run while DMAs/casts are in flight)
            warm = singles.tile([128, 256], BF16, tag="warm")
            nc.vector.memset(warm, 0.0)
            wslot = sc_pool.tile([128, 2, 512], FP32, tag="slot", name="wslot")
            for _ in range(WARM_MMS):
                nc.tensor.matmul(
                    wslot[:, 0, 0:256], tri01, warm, start=True, stop=True
                )

            # ---- per-head tiles ----
            def alloc_head():
                return {
                    "natQ": nat_pool.tile([128, NT, D], FP32, tag="natQ", name="natQ"),
                    "natK": nat_pool.tile([128, NT, D], FP32, tag="natK", name="natK"),
                    "natV": nat_pool.tile([128, NT, D], FP32, tag="natV", name="natV"),
                    # [K-half (8x64) | Q-dup-half (8x2x64)], one tile per q-half
                    # (separate tiles so transposes never dep on the other
                    # half's casts)
                    "bfkq0": bf_pool.tile([128, 1536], BF16, tag="bfkq0", name="bfkq0"),
                    "bfkq1": bf_pool.tile([128, 1536], BF16, tag="bfkq1", name="bfkq1"),
                    # transposed: [:, 0:4, :] = kslab pairs, [:, 4:12, :] = qt
                    "trs0": slab_pool.tile([128, 12, 128], BF16, tag="trs0", name="trs0"),
                    "trs1": slab_pool.tile([128, 12, 128], BF16, tag="trs1", name="trs1"),
                    # V padded to 128 cols: 0:64 = V, 64 = ones, 65:128 junk
                    "vaug": v_pool.tile([128, NT, 128], BF16, tag="vaug", name="vaug"),
                }

            def emit_load_qk(h, hd, pieces, eng=None):
                eng = eng or nc.gpsimd
                qsrc = q_d[h].rearrange("(t p) d -> p t d", p=128)
                ksrc = k_d[h].rearrange("(t p) d -> p t d", p=128)
                for a, b in pieces:
                    eng.dma_start(out=hd["natQ"][:, a:b, :], in_=qsrc[:, a:b, :])
                    eng.dma_start(out=hd["natK"][:, a:b, :], in_=ksrc[:, a:b, :])

            def emit_load_v(h, hd, eng=None):
                eng = eng or nc.gpsimd
                vsrc = v_d[h].rearrange("(t p) d -> p t d", p=128)
                eng.dma_start(out=hd["natV"], in_=vsrc)

            def emit_cast_half(hd, ph, pieces=((0, 8),)):
                bfkq = hd[f"bfkq{ph}"]
                for a, b in pieces:  # a, b local tile idx within the half
                    sl = slice(8 * ph + a, 8 * ph + b)
                    kv = bfkq[:, 64 * a : 64 * b].rearrange(
                        "p (t d) -> p t d", d=D
                    )
                    nc.vector.tensor_copy(kv, hd["natK"][:, sl, :])
                    T["dve"] += ((b - a) * 64 / 2 + 140) * 1.0417
                    qv = bfkq[:, 512 + 128 * a : 512 + 128 * b].rearrange(
                        "p (t c d) -> p t c d", c=2, d=D
                    )
                    qin = (
                        hd["natQ"][:, sl, :]
                        .unsqueeze(2)
                        .broadcast_to([128, b - a, 2, D])
                    )
                    nc.vector.tensor_copy(qv, qin)
                    T["dve"] += ((b - a) * 128 / 2 + 140) * 1.0417

            def emit_transpose_half(hd, ph, pieces=None):
                bfkq, trs = hd[f"bfkq{ph}"], hd[f"trs{ph}"]
                if pieces is None:
                    nc.sync.dma_start_transpose(out=trs, in_=bfkq)
                    return
                for a, b in pieces:  # K part then Q part per piece
                    nc.sync.dma_start_transpose(
                        out=trs[:, a // 2 : b // 2, :],
                        in_=bfkq[:, 64 * a : 64 * b],
                    )
                    nc.sync.dma_start_transpose(
                        out=trs[:, 4 + a : 4 + b, :],
                        in_=bfkq[:, 512 + 128 * a : 512 + 128 * b],
                    )

            def emit_cast_v(hd):
                nc.vector.tensor_copy(hd["vaug"][:, :, 0:D], hd["natV"])
                nc.vector.memset(hd["vaug"][:, :, D : D + 1], 1.0)
                T["dve"] += (NT * D / 2 + 140) * 1.0417 + 80

            def kslab_ap(hd, rows, kj):
                j = kj // 2
                return hd[f"trs{j // 4}"][rows : rows + 64, j % 4, :]

            def qt_ap(hd, rows, ca, cb):
                hfq = ca // 1024
                trs_f = hd[f"trs{hfq}"].rearrange("p b c -> p (b c)")
                return trs_f[
                    rows : rows + 64,
                    512 + ca - 1024 * hfq : 512 + cb - 1024 * hfq,
                ]

            # ---- eviction units (static greedy ACT/DVE balance) ----
            def evict_unit(slot, ptile, cols, diag):
                el = 2 * cols
                if T["act"] + _act_cost(el) <= T["dve"] + _dve_cost(el):
                    T["act"] += _act_cost(el)
                    nc.scalar.activation(
                        ptile[:, :, 0:cols],
                        slot[:, :, 0:cols],
                        mybir.ActivationFunctionType.Exp,
                        scale=0.125,
                    )
                    if diag:
                        nc.gpsimd.tensor_mul(
                            ptile[:, :, 0:256], ptile[:, :, 0:256], m2
                        )
                        T["gps"] += 900
                else:
                    T["dve"] += _dve_cost(el)
                    p16 = ptile.bitcast(I16)
                    if diag:
                        nc.vector.scalar_tensor_tensor(
                            out=p16[:, :, 0:cols],
                            in0=slot[:, :, 0:cols],
                            scalar=EXP_A,
                            in1=mb[:, :, 0:cols],
                            op0=AluOpType.mult,
                            op1=AluOpType.add,
                        )
                    else:
                        nc.vector.tensor_scalar(
                            out=p16[:, :, 0:cols],
                            in0=slot[:, :, 0:cols],
                            scalar1=EXP_A,
                            scalar2=EXP_B,
                            op0=AluOpType.mult,
                            op1=AluOpType.add,
                        )

            def bfo_unit(dst, src):
                el = 512
                if T["act"] + _act_cost(el) + 400 <= T["dve"] + _dve_cost(el):
                    T["act"] += _act_cost(el)
                    nc.scalar.copy(dst, src)
                else:
                    T["dve"] += _dve_cost(el)
                    nc.vector.tensor_copy(dst, src)

            # ---- one (head, half) of compute ----
            def emit_half(h, hd, hf, pending_fin):
                q0 = 1024 * hf
                q1 = q0 + 1024
                kj_hi = 8 * (hf + 1)
                last_kj = [
                    max(
                        kj
                        for kj in range(kj_hi)
                        if max(q0, 128 * kj) < q0 + 512 * (b + 1)
                    )
                    for b in range(2)
                ]

                outps = ops_pool.tile([128, 2, 512], FP32, tag="outps")
                outps_f = outps.rearrange("p a b -> p (a b)")

                def emit_pv(pair, qas, chunks):
                    for lane, (kj, qa) in enumerate(zip(pair, qas)):
                        for ca, cb, ptile in chunks:
                            lo = max(ca, qa)
                            while lo < cb:
                                hi = min(cb, q0 + 512 * ((lo - q0) // 512 + 1))
                                bk = (lo - q0) // 512
                                nc.tensor.matmul(
                                    outps_f[:, lo - q0 : hi - q0],
                                    hd["vaug"][:, kj, :],
                                    ptile[:, lane, lo - ca : hi - ca],
                                    start=(kj == 0),
                                    stop=(kj == last_kj[bk]),
                                )
                                lo = hi

                pending = []
                for pj in range(kj_hi // 2):
                    pair = (2 * pj, 2 * pj + 1)
                    qas = [max(q0, 128 * kj) for kj in pair]
                    diag0 = 128 * pair[0] >= q0  # first chunk contains diags
                    chunks = []
                    for ca in range(qas[0], q1, 512):
                        cb = min(ca + 512, q1)
                        cols = cb - ca
                        slot = sc_pool.tile(
                            [128, 2, 512], FP32, tag="slot", name="slot"
                        )
                        for lane, (kj, qa) in enumerate(zip(pair, qas)):
                            lo = max(ca, qa)
                            if lo >= cb:
                                continue
                            rows = (kj % 2) * 64
                            nc.tensor.matmul(
                                slot[:, lane, lo - ca : cols],
                                kslab_ap(hd, rows, kj),
                                qt_ap(hd, rows, lo, cb),
                                start=True,
                                stop=True,
                            )
                        ptile = pt_pool.tile(
                            [128, 2, 512], BF16, tag="ptile", name="ptile"
                        )
                        evict_unit(slot, ptile, cols, diag0 and ca == qas[0])
                        chunks.append((ca, cb, ptile))
                    pending.append((pair, qas, chunks))
                    # PV lags TWO pairs: PV(j) head-of-line-blocks the PE
                    # FIFO on evict(j); with lag 2 the eviction completes
                    # while QK(j+1)/QK(j+2) stream.
                    if pj >= 2:
                        emit_pv(*pending.pop(0))
                for args in pending:
                    emit_pv(*args)
                while len(pending_fin) > 1:
                    pending_fin.pop(0)()

                # ---- epilogue, part 1: release PSUM + start the transpose.
                # rec/fo/out are DEFERRED a full half (returned as a closure)
                # so the DVE FIFO never head-of-line blocks on the onat DMA.
                bfo = ep_pool.tile([80, 2, 512], BF16, tag="bfo")
                bfo_f = bfo.rearrange("p a b -> p (a b)")
                bfo_unit(bfo[:, 0, :], outps_f[0:80, 0:512])
                bfo_unit(bfo[:, 1, :], outps_f[0:80, 512:1024])
                onat = ep_pool.tile([128, 8, 80], BF16, tag="onat")
                nc.sync.dma_start_transpose(out=onat, in_=bfo_f)

                def finish():
                    rec = ep_pool.tile([128, 8], FP32, tag="rec")
                    nc.vector.reciprocal(rec, onat[:, :, D])
                    T["dve"] += 210
                    fo = ep_pool.tile([128, 8, D], FP32, tag="fo")
                    nc.vector.tensor_tensor(
                        out=fo,
                        in0=onat[:, :, 0:D],
                        in1=rec.unsqueeze(2).broadcast_to([128, 8, D]),
                        op=AluOpType.mult,
                    )
                    T["dve"] += 680
                    odst = o_d[h].rearrange("(t p) d -> p t d", p=128)
                    nc.sync.dma_start(
                        out=odst[:, 8 * hf : 8 * hf + 8, :], in_=fo
                    )

                return finish

            # ---- schedule: loads run TWO heads ahead (gpsimd SWDGE ring),
            # casts+transposes ONE head ahead -> every cross-engine dep has
            # a full head (~25us) of slack, so no queue head-of-line blocks.
            hd = [None] * HPC
            hd[0] = alloc_head()
            # head 0/1 loads on the (idle at startup) Sync ring, critical
            # first pieces first
            emit_load_qk(0, hd[0], [(0, 4), (4, 8)], eng=nc.sync)
            emit_load_v(0, hd[0], eng=nc.sync)
            emit_load_qk(0, hd[0], [(8, 16)], eng=nc.sync)
            if HPC > 1:
                hd[1] = alloc_head()
                emit_load_qk(1, hd[1], [(0, 8), (8, 16)], eng=nc.sync)
                emit_load_v(1, hd[1], eng=nc.sync)
            emit_cast_half(hd[0], 0, pieces=((0, 4), (4, 8)))
            emit_transpose_half(hd[0], 0, pieces=((0, 4), (4, 8)))
            emit_cast_half(hd[0], 1)
            emit_transpose_half(hd[0], 1)
            emit_cast_v(hd[0])

            pending_fin = []
            for h in range(HPC):
                if h + 1 < HPC:
                    nxt = hd[h + 1]
                    emit_cast_half(nxt, 0)
                    emit_transpose_half(nxt, 0)
                    emit_cast_half(nxt, 1)
                    emit_transpose_half(nxt, 1)
                    emit_cast_v(nxt)
                order = (0, 1) if h + 1 < HPC else (1, 0)
                fin = emit_half(h, hd[h], order[0], pending_fin)
                pending_fin.append(fin)
                if h + 2 < HPC:
                    hd[h + 2] = alloc_head()
                    emit_load_qk(h + 2, hd[h + 2], [(0, 8), (8, 16)])
                    emit_load_v(h + 2, hd[h + 2])
                fin = emit_half(h, hd[h], order[1], pending_fin)
                pending_fin.append(fin)
            for fin in pending_fin:
                fin()

    nc.compile()
    import os

    if os.environ.get("BASS_DEBUG_BALANCE"):
        print(f"balance estimate/core: {T}")
    return nc


_NC = None


def _get_nc():
    global _NC
    if _NC is None:
        _NC = build_attention()
    return _NC


def kernel(query, key, value):
    nc = _get_nc()
    q = np.ascontiguousarray(query, dtype=np.float32).reshape(B * H, S, D)
    k = np.ascontiguousarray(key, dtype=np.float32).reshape(B * H, S, D)
    v = np.ascontiguousarray(value, dtype=np.float32).reshape(B * H, S, D)
    in_maps = [
        {
            "query": q[i * HPC : (i + 1) * HPC],
            "key": k[i * HPC : (i + 1) * HPC],
            "value": v[i * HPC : (i + 1) * HPC],
        }
        for i in range(N_CORES)
    ]
    res = run_bass_kernel_spmd(nc, in_maps, core_ids=list(range(N_CORES)))
    out = np.concatenate([res.results[i]["out"] for i in range(N_CORES)], axis=0)
    return out.reshape(B, H, S, D)


# revision 12
# speedup vs baseline: 1.3076x; 1.3076x over previous
"""Causal multi-head attention (B=2, H=16, S=2048, D=64, fp32 I/O) on 8 TRN2
NeuronCores.

Sharding: batch*heads (32 units) split 4-per-core — embarrassingly parallel,
no collectives.

Per-core kernel v2 (bf16 compute, fp32 PSUM accumulation):
  - scores computed TRANSPOSED: scoresT[k, q] = K_blk @ Q^T; softmax
    numerators P^T feed P@V directly with V (+ones column for the softmax
    denominator) as the stationary operand, PADDED to 128 columns so the
    compiler's fast-weight-load (FWL) path triggers.
  - PSUM->SBUF score eviction is split across BOTH ScalarE and VectorE:
    ScalarE chunks use the exact Exp activation; VectorE chunks use a
    1-instruction Schraudolph exp (int16(x*A+B) bits reinterpreted as bf16,
    ~1.8% avg rel err on a ~25-40%% fraction of elements — global output
    rel err stays ~1e-2 < 2e-2). A static greedy balancer assigns chunks.
  - causal diag-block masking: VectorE chunks fuse the mask into the
    Schraudolph affine via a per-chunk bias tensor (-30k => ~-1e-32 weights);
    ScalarE chunks get a single merged GpSimd 0/1-mask multiply.
  - PSUM: 3 double-bank score slots + 1 double-bank out^T accumulator
    (deeper score pipeline hides PE latency; out^T is evicted per-bank
    early enough that single-buffering doesn't stall the next half).
  - K^T/Q^T built via ONE blocked DMA-transpose per q-half from a combined
    [K|Q-duplicated] bf16 buffer; next head's casts+transposes are emitted
    during the current head's second half so ScalarE never idles at head
    boundaries. A dozen dummy matmuls at startup pre-warm the PE HAM clock.
"""

import numpy as np

import concourse.bass as bass
import concourse.mybir as mybir
import concourse.tile as tile
from concourse import bacc
from concourse.bass_utils import run_bass_kernel_spmd
from concourse.masks import make_upper_triangular
from concourse.alu_op_type import AluOpType

B, H, S, D = 2, 16, 2048, 64
N_CORES = 8
HPC = (B * H) // N_CORES  # heads per core
NT = S // 128  # 16 k/q blocks of 128
FP32 = mybir.dt.float32
BF16 = mybir.dt.bfloat16
I16 = mybir.dt.int16

LOG2E = 1.4426950408889634
EXP_A = 128.0 * LOG2E / 8.0  # folds softmax scale 1/sqrt(64) into the affine
EXP_B = 127.0 * 128.0 - 5.5
MASK_NEG = -30720.0

SC_BUFS = 3
OPS_BUFS = 1
WARM_MMS = 12

# engine cost models (ns) for the static eviction balancer
def _act_cost(el):
    return (el + 230) * 0.8333


def _dve_cost(el):
    return (el + 140) * 1.0417


def build_attention():
    nc = bacc.Bacc("TRN2", target_bir_lowering=False)
    q_d = nc.dram_tensor("query", [HPC, S, D], FP32, kind="ExternalInput")
    k_d = nc.dram_tensor("key", [HPC, S, D], FP32, kind="ExternalInput")
    v_d = nc.dram_tensor("value", [HPC, S, D], FP32, kind="ExternalInput")
    o_d = nc.dram_tensor("out", [HPC, S, D], FP32, kind="ExternalOutput")

    T = {"act": 0.0, "dve": 0.0, "gps": 0.0}

    with tile.TileContext(nc) as tc:
        with (
            tc.tile_pool(name="singles", bufs=1) as singles,
            tc.tile_pool(name="nat", bufs=2) as nat_pool,
            tc.tile_pool(name="bf", bufs=2) as bf_pool,
            tc.tile_pool(name="slab", bufs=2) as slab_pool,
            tc.tile_pool(name="vp", bufs=2) as v_pool,
            tc.tile_pool(name="pt", bufs=7) as pt_pool,
            tc.tile_pool(name="ep", bufs=4) as ep_pool,
            tc.tile_pool(name="sc", bufs=SC_BUFS, space="PSUM") as sc_pool,
            tc.tile_pool(name="ops", bufs=OPS_BUFS, space="PSUM") as ops_pool,
        ):
            # ---- one-time masks ----
            # 0/1 keep-mask for ScalarE diag chunks (partition = k, free = q)
            tri01 = singles.tile([128, 128], BF16, tag="tri01")
            make_upper_triangular(nc, tri01, val=1.0, diag=True)
            # merged 2-lane mask: lane0 diag at cols 0:128, lane1 at 128:256
            m2 = singles.tile([128, 2, 256], BF16, tag="m2")
            nc.gpsimd.memset(m2, 1.0)
            nc.gpsimd.tensor_copy(m2[:, 0, 0:128], tri01)
            nc.gpsimd.tensor_copy(m2[:, 1, 128:256], tri01)
            # VectorE fast-exp bias-with-mask: B where keep, -30k where masked
            mbt = singles.tile([128, 128], FP32, tag="mbt")
            nc.gpsimd.memset(mbt, MASK_NEG)
            nc.gpsimd.affine_select(
                out=mbt,
                in_=mbt,
                compare_op=mybir.AluOpType.is_gt,
                fill=EXP_B,
                base=0,
                pattern=[[-1, 128]],
                channel_multiplier=1,
            )
            mb = singles.tile([128, 2, 512], FP32, tag="mb")
            nc.vector.memset(mb, EXP_B)
            nc.vector.tensor_copy(mb[:, 0, 0:128], mbt)
            nc.vector.tensor_copy(mb[:, 1, 128:256], mbt)

            # PE HAM warm-up: dummy matmuls so the clock gate opens before
            # the first real QK^T (they run while DMAs/casts are in flight)
            warm = singles.tile([128, 256], BF16, tag="warm")
            nc.vector.memset(warm, 0.0)
            wslot = sc_pool.tile([128, 2, 512], FP32, tag="slot", name="wslot")
            for _ in range(WARM_MMS):
                nc.tensor.matmul(
                    wslot[:, 0, 0:256], tri01, warm, start=True, stop=True
                )

            # ---- per-head tiles ----
            def alloc_head():
                return {
                    "natQ": nat_pool.tile([128, NT, D], FP32, tag="natQ", name="natQ"),
                    "natK": nat_pool.tile([128, NT, D], FP32, tag="natK", name="natK"),
                    "natV": nat_pool.tile([128, NT, D], FP32, tag="natV", name="natV"),
                    # [K-half (8x64) | Q-dup-half (8x2x64)] per q-half
                    "bfkq": bf_pool.tile([128, 2, 1536], BF16, tag="bfkq", name="bfkq"),
                    # transposed: [:, ph, 0:4, :] = kslab pairs, [:, ph, 4:12, :] = qt
                    "trs": slab_pool.tile([128, 2, 12, 128], BF16, tag="trs", name="trs"),
                    # V padded to 128 cols: 0:64 = V, 64 = ones, 65:128 junk
                    "vaug": v_pool.tile([128, NT, 128], BF16, tag="vaug", name="vaug"),
                }

            def emit_load_qk(h, hd, pieces):
                qsrc = q_d[h].rearrange("(t p) d -> p t d", p=128)
                ksrc = k_d[h].rearrange("(t p) d -> p t d", p=128)
                for a, b in pieces:
                    nc.sync.dma_start(out=hd["natQ"][:, a:b, :], in_=qsrc[:, a:b, :])
                    nc.sync.dma_start(out=hd["natK"][:, a:b, :], in_=ksrc[:, a:b, :])

            def emit_load_v(h, hd):
                vsrc = v_d[h].rearrange("(t p) d -> p t d", p=128)
                nc.sync.dma_start(out=hd["natV"], in_=vsrc)

            def emit_cast_half(hd, ph, pieces=((0, 8),)):
                bfkq = hd["bfkq"]
                for a, b in pieces:  # a, b local tile idx within the half
                    sl = slice(8 * ph + a, 8 * ph + b)
                    kv = bfkq[:, ph, 64 * a : 64 * b].rearrange(
                        "p (t d) -> p t d", d=D
                    )
                    nc.vector.tensor_copy(kv, hd["natK"][:, sl, :])
                    T["dve"] += ((b - a) * 64 / 2 + 140) * 1.0417
                    qv = bfkq[:, ph, 512 + 128 * a : 512 + 128 * b].rearrange(
                        "p (t c d) -> p t c d", c=2, d=D
                    )
                    qin = (
                        hd["natQ"][:, sl, :]
                        .unsqueeze(2)
                        .broadcast_to([128, b - a, 2, D])
                    )
                    nc.vector.tensor_copy(qv, qin)
                    T["dve"] += ((b - a) * 128 / 2 + 140) * 1.0417

            def emit_transpose_half(hd, ph, pieces=None):
                bfkq, trs = hd["bfkq"], hd["trs"]
                if pieces is None:
                    nc.sync.dma_start_transpose(
                        out=trs[:, ph, :, :], in_=bfkq[:, ph, :]
                    )
                    return
                for a, b in pieces:  # K part then Q part per piece
                    nc.sync.dma_start_transpose(
                        out=trs[:, ph, a // 2 : b // 2, :],
                        in_=bfkq[:, ph, 64 * a : 64 * b],
                    )
                    nc.sync.dma_start_transpose(
                        out=trs[:, ph, 4 + a : 4 + b, :],
                        in_=bfkq[:, ph, 512 + 128 * a : 512 + 128 * b],
                    )

            def emit_cast_v(hd):
                nc.vector.tensor_copy(hd["vaug"][:, :, 0:D], hd["natV"])
                nc.vector.memset(hd["vaug"][:, :, D : D + 1], 1.0)
                T["dve"] += (NT * D / 2 + 140) * 1.0417 + 80

            def kslab_ap(hd, rows, kj):
                j = kj // 2
                return hd["trs"][rows : rows + 64, j // 4, j % 4, :]

            def qt_ap(hd, rows, ca, cb):
                trs_f = hd["trs"].rearrange("p a b c -> p (a b c)")
                hfq = ca // 1024
                base = hfq * 1536 + 512
                return trs_f[
                    rows : rows + 64,
                    base + ca - 1024 * hfq : base + cb - 1024 * hfq,
                ]

            # ---- eviction units (static greedy ACT/DVE balance) ----
            def evict_unit(slot, ptile, cols, diag):
                el = 2 * cols
                if T["act"] + _act_cost(el) <= T["dve"] + _dve_cost(el):
                    T["act"] += _act_cost(el)
                    nc.scalar.activation(
                        ptile[:, :, 0:cols],
                        slot[:, :, 0:cols],
                        mybir.ActivationFunctionType.Exp,
                        scale=0.125,
                    )
                    if diag:
                        nc.gpsimd.tensor_mul(
                            ptile[:, :, 0:256], ptile[:, :, 0:256], m2
                        )
                        T["gps"] += 900
                else:
                    T["dve"] += _dve_cost(el)
                    p16 = ptile.bitcast(I16)
                    if diag:
                        nc.vector.scalar_tensor_tensor(
                            out=p16[:, :, 0:cols],
                            in0=slot[:, :, 0:cols],
                            scalar=EXP_A,
                            in1=mb[:, :, 0:cols],
                            op0=AluOpType.mult,
                            op1=AluOpType.add,
                        )
                    else:
                        nc.vector.tensor_scalar(
                            out=p16[:, :, 0:cols],
                            in0=slot[:, :, 0:cols],
                            scalar1=EXP_A,
                            scalar2=EXP_B,
                            op0=AluOpType.mult,
                            op1=AluOpType.add,
                        )

            def bfo_unit(dst, src):
                el = 512
                if T["act"] + _act_cost(el) <= T["dve"] + _dve_cost(el):
                    T["act"] += _act_cost(el)
                    nc.scalar.copy(dst, src)
                else:
                    T["dve"] += _dve_cost(el)
                    nc.vector.tensor_copy(dst, src)

            # ---- one (head, half) of compute ----
            def emit_half(h, hd, hf):
                q0 = 1024 * hf
                q1 = q0 + 1024
                kj_hi = 8 * (hf + 1)
                last_kj = [
                    max(
                        kj
                        for kj in range(kj_hi)
                        if max(q0, 128 * kj) < q0 + 512 * (b + 1)
                    )
                    for b in range(2)
                ]

                outps = ops_pool.tile([128, 2, 512], FP32, tag="outps")
                outps_f = outps.rearrange("p a b -> p (a b)")

                def emit_pv(pair, qas, chunks):
                    for lane, (kj, qa) in enumerate(zip(pair, qas)):
                        for ca, cb, ptile in chunks:
                            lo = max(ca, qa)
                            while lo < cb:
                                hi = min(cb, q0 + 512 * ((lo - q0) // 512 + 1))
                                bk = (lo - q0) // 512
                                nc.tensor.matmul(
                                    outps_f[:, lo - q0 : hi - q0],
                                    hd["vaug"][:, kj, :],
                                    ptile[:, lane, lo - ca : hi - ca],
                                    start=(kj == 0),
                                    stop=(kj == last_kj[bk]),
                                )
                                lo = hi

                pending = []
                for pj in range(kj_hi // 2):
                    pair = (2 * pj, 2 * pj + 1)
                    qas = [max(q0, 128 * kj) for kj in pair]
                    diag0 = 128 * pair[0] >= q0  # first chunk contains diags
                    chunks = []
                    for ca in range(qas[0], q1, 512):
                        cb = min(ca + 512, q1)
                        cols = cb - ca
                        slot = sc_pool.tile(
                            [128, 2, 512], FP32, tag="slot", name="slot"
                        )
                        for lane, (kj, qa) in enumerate(zip(pair, qas)):
                            lo = max(ca, qa)
                            if lo >= cb:
                                continue
                            rows = (kj % 2) * 64
                            nc.tensor.matmul(
                                slot[:, lane, lo - ca : cols],
                                kslab_ap(hd, rows, kj),
                                qt_ap(hd, rows, lo, cb),
                                start=True,
                                stop=True,
                            )
                        ptile = pt_pool.tile(
                            [128, 2, 512], BF16, tag="ptile", name="ptile"
                        )
                        evict_unit(slot, ptile, cols, diag0 and ca == qas[0])
                        chunks.append((ca, cb, ptile))
                    pending.append((pair, qas, chunks))
                    if pj >= 1:
                        emit_pv(*pending.pop(0))
                for args in pending:
                    emit_pv(*args)

                # ---- epilogue ----
                bfo = ep_pool.tile([80, 2, 512], BF16, tag="bfo")
                bfo_f = bfo.rearrange("p a b -> p (a b)")
                bfo_unit(bfo[:, 0, :], outps_f[0:80, 0:512])
                bfo_unit(bfo[:, 1, :], outps_f[0:80, 512:1024])
                onat = ep_pool.tile([128, 8, 80], BF16, tag="onat")
                nc.sync.dma_start_transpose(out=onat, in_=bfo_f)
                rec = ep_pool.tile([128, 8], FP32, tag="rec")
                nc.vector.reciprocal(rec, onat[:, :, D])
                T["dve"] += 210
                fo = ep_pool.tile([128, 8, D], FP32, tag="fo")
                nc.vector.tensor_tensor(
                    out=fo,
                    in0=onat[:, :, 0:D],
                    in1=rec.unsqueeze(2).broadcast_to([128, 8, D]),
                    op=AluOpType.mult,
                )
                T["dve"] += 680
                odst = o_d[h].rearrange("(t p) d -> p t d", p=128)
                nc.sync.dma_start(
                    out=odst[:, 8 * hf : 8 * hf + 8, :], in_=fo
                )

            # ---- schedule ----
            hd = [None] * HPC
            hd[0] = alloc_head()
            emit_load_qk(0, hd[0], [(0, 4), (4, 8)])
            emit_load_v(0, hd[0])
            emit_load_qk(0, hd[0], [(8, 16)])
            emit_cast_half(hd[0], 0, pieces=((0, 4), (4, 8)))
            emit_transpose_half(hd[0], 0, pieces=((0, 4), (4, 8)))
            emit_cast_half(hd[0], 1)
            emit_transpose_half(hd[0], 1)
            emit_cast_v(hd[0])

            for h in range(HPC):
                order = (0, 1) if h + 1 < HPC else (1, 0)
                for idx, hf in enumerate(order):
                    if idx == 0 and h + 1 < HPC:
                        hd[h + 1] = alloc_head()
                        emit_load_qk(h + 1, hd[h + 1], [(0, 8), (8, 16)])
                        emit_load_v(h + 1, hd[h + 1])
                    if idx == 1 and h + 1 < HPC:
                        nxt = hd[h + 1]
                        emit_cast_half(nxt, 0)
                        emit_transpose_half(nxt, 0)
                        emit_cast_half(nxt, 1)
                        emit_transpose_half(nxt, 1)
                        emit_cast_v(nxt)
                    emit_half(h, hd[h], hf)

    nc.compile()
    import os

    if os.environ.get("BASS_DEBUG_BALANCE"):
        print(f"balance estimate/core: {T}")
    return nc


_NC = None


def _get_nc():
    global _NC
    if _NC is None:
        _NC = build_attention()
    return _NC


def kernel(query, key, value):
    nc = _get_nc()
    q = np.ascontiguousarray(query, dtype=np.float32).reshape(B * H, S, D)
    k = np.ascontiguousarray(key, dtype=np.float32).reshape(B * H, S, D)
    v = np.ascontiguousarray(value, dtype=np.float32).reshape(B * H, S, D)
    in_maps = [
        {
            "query": q[i * HPC : (i + 1) * HPC],
            "key": k[i * HPC : (i + 1) * HPC],
            "value": v[i * HPC : (i + 1) * HPC],
        }
        for i in range(N_CORES)
    ]
    res = run_bass_kernel_spmd(nc, in_maps, core_ids=list(range(N_CORES)))
    out = np.concatenate([res.results[i]["out"] for i in range(N_CORES)], axis=0)
    return out.reshape(B, H, S, D)
